# revision 27
# baseline (speedup 1.0000x reference)
"""Multi-head attention (B=2, S=2048, D=1024, H=16, DK=64) with RoPE and
causal masking, sharded over 8 NeuronCores as (batch x head-group):
core c handles batch c//4 and heads 4*(c%4) .. 4*(c%4)+4.

Per-core dataflow (all layouts chosen so no on-device transposes are needed):
  - host pre-transposes activations X^T [D, S] and weight slices.
  - projections produce Q^T/K^T in an "A layout" per 128-partition tile
    ([h0e|h1e|h0o|h1o]: RoPE even/odd dims grouped) via PE matmul,
    evicted from PSUM with fused bias add (DVE tensor_scalar_add).
  - RoPE runs full-width DVE ops on the A tiles and re-packs into the
    "B layout" ([h0e'|h0o'|h1e'|h1o']: head-contiguous, 64 dims/head).
  - scores_t[k,q] = K_B.T @ Q_B per (qtile 512, kblock 128, head), K_c=64.
    Softmax without max-subtraction (scores ~ N(0,1), safe in fp32):
    P = exp(0.125*s + addpat) where addpat is 0/-1e30; causal blocks above
    the diagonal are skipped entirely.
  - AV: lhsT = [V | ones] (M=65) stationary, rhs = P_t moving; PSUM
    accumulates over kblocks; row 64 gives the softmax denominator l.
  - normalize: O^T = AV[0:64] * broadcast(1/l) (broadcast via K_c=1 matmul).
  - output projection: partial^T[j,s] = woT.T @ O^T; host sums the 4
    partials per batch, transposes, and adds bo.

The whole thing is emitted as one software pipeline over the 4 q s-tiles:
projections(st) -> RoPE(st) -> attention(qt=st) -> out-proj(st), so Tile
overlaps DMA/PE/DVE/ACT across phases.
"""
import numpy as np
import ml_dtypes

BF = ml_dtypes.bfloat16
B, S, D, H, DK = 2, 2048, 1024, 16, 64
NCORES = 8
HPC = 4            # heads per core
DH = HPC * DK      # 256 local head dims
QT = 512           # q tile (free dim of scores matmul)
KB = 128           # k block (partition dim of scores)
NQT = S // QT      # 4
NKB = S // KB      # 16
NDB = D // 128     # 8 d-blocks for projections
NEG = -1.0e30
NEG8 = -1.0e31   # pre-scale mask value: *0.125 -> -1.25e30

_cache = {}


def _build_nc(causal: bool):
    from contextlib import ExitStack
    import concourse.bass as bass
    import concourse.tile as tile
    from concourse import bacc, mybir
    from concourse._compat import with_exitstack

    F32 = mybir.dt.float32
    BF16 = mybir.dt.bfloat16
    AF = mybir.ActivationFunctionType
    OP = mybir.AluOpType

    nc = bacc.Bacc(None, target_bir_lowering=False, debug=False)

    xqT_d = nc.dram_tensor("xqT", [D, S], BF16, kind="ExternalInput")
    xkT_d = nc.dram_tensor("xkT", [D, S], BF16, kind="ExternalInput")
    xvT_d = nc.dram_tensor("xvT", [D, S], BF16, kind="ExternalInput")
    wqT_d = nc.dram_tensor("wqT", [D, DH], BF16, kind="ExternalInput")
    wkT_d = nc.dram_tensor("wkT", [D, DH], BF16, kind="ExternalInput")
    wvT_d = nc.dram_tensor("wvT", [D, DH], BF16, kind="ExternalInput")
    bqA_d = nc.dram_tensor("bqA", [DH], F32, kind="ExternalInput")
    bkA_d = nc.dram_tensor("bkA", [DH], F32, kind="ExternalInput")
    bv_d = nc.dram_tensor("bv", [DH], F32, kind="ExternalInput")
    woT_d = nc.dram_tensor("woT", [DH, D], BF16, kind="ExternalInput")
    sin_d = nc.dram_tensor("sin4", [128, S], F32, kind="ExternalInput")
    cos_d = nc.dram_tensor("cos4", [128, S], F32, kind="ExternalInput")
    if causal:
        pat_d = nc.dram_tensor("addtri", [KB, KB], F32, kind="ExternalInput")
    else:
        pat_d = nc.dram_tensor("amaskT", [S, S], F32, kind="ExternalInput")
    eye4_d = nc.dram_tensor("eye4", [4, 4, 64], F32, kind="ExternalInput")
    out_d = nc.dram_tensor("outT", [D, S], F32, kind="ExternalOutput")

    @with_exitstack
    def emit(ctx: ExitStack, tc: tile.TileContext):
        nc = tc.nc
        singles = ctx.enter_context(tc.tile_pool(name="singles", bufs=1))
        xpool = ctx.enter_context(tc.tile_pool(name="x", bufs=3))
        ropet = ctx.enter_context(tc.tile_pool(name="ropet", bufs=2))
        bpool = ctx.enter_context(tc.tile_pool(name="bpool", bufs=1))
        ppool = ctx.enter_context(tc.tile_pool(name="ppool", bufs=8))
        opool = ctx.enter_context(tc.tile_pool(name="opool", bufs=3))
        if not causal:
            ampool = ctx.enter_context(tc.tile_pool(name="ampool", bufs=3))

        # "big" slots are 2 banks each ([128,2,QT]); proj psum pairs and
        # score groups rotate through them. av holds AV accumulators and the
        # tiny broadcast matmuls.
        ps_big = ctx.enter_context(tc.tile_pool(name="ps_big", bufs=2, space="PSUM"))
        ps_av = ctx.enter_context(tc.tile_pool(name="ps_av", bufs=4, space="PSUM"))

        # ---- constants on the gpsimd queue; weights go on sync, each
        # emitted right before its first consumer ----
        wq = singles.tile([128, NDB, DH], BF16)
        wk = singles.tile([128, NDB, DH], BF16)
        wv = singles.tile([128, NDB, DH], BF16)
        sin4 = singles.tile([128, S], F32)
        cos4 = singles.tile([128, S], F32)
        nc.gpsimd.dma_start(out=sin4, in_=sin_d[:])
        nc.gpsimd.dma_start(out=cos4, in_=cos_d[:])
        eye4 = singles.tile([4, 4, 64], F32)
        nc.gpsimd.dma_start(out=eye4, in_=eye4_d[:])
        if causal:
            addtri = singles.tile([KB, KB], F32)
            nc.gpsimd.dma_start(out=addtri, in_=pat_d[:])

        bq_sb = singles.tile([128, 2], F32)
        bk_sb = singles.tile([128, 2], F32)
        nc.gpsimd.dma_start(out=bq_sb, in_=bqA_d.rearrange("(t p) -> p t", p=128))
        nc.gpsimd.dma_start(out=bk_sb, in_=bkA_d.rearrange("(t p) -> p t", p=128))
        bv_bc = singles.tile([128, DH], F32)
        nc.gpsimd.dma_start(
            out=bv_bc,
            in_=bass.AP(tensor=bv_d[:].tensor, offset=0, ap=[[0, 128], [1, DH]]),
        )

        woT_sb = singles.tile([128, 2, D], BF16)
        nc.gpsimd.dma_start(
            out=woT_sb, in_=woT_d.rearrange("(n p) m -> p n m", p=128))

        # persistent activations (post-RoPE Q/K and attention output, T layout)
        qb = [bpool.tile([128, S], BF16, tag=f"qb{t}", name=f"qb{t}") for t in range(2)]
        kb_t = [bpool.tile([128, S], BF16, tag=f"kbt{t}", name=f"kbt{t}") for t in range(2)]
        oT = [bpool.tile([128, S], BF16, tag=f"oT{t}", name=f"oT{t}") for t in range(2)]
        v_sb = singles.tile([128, NKB, HPC, DK + 1], BF16)
        nc.vector.memset(v_sb[:, :, :, DK:DK + 1], 1.0)

        def make_qkproj(xd, ws, bias_sb, dst, st):
            """Returns (pairs, finish): `pairs` are per-d-block closures each
            emitting the E+O matmuls (interleaved into the attention stream
            as PE keep-alive filler); `finish` emits bias+RoPE (STT) and the
            stream-order scatter DMA into the head-contiguous dst tiles."""
            xt = [xpool.tile([128, NDB // 2, QT], BF16, tag="xtA", name="xtA"),
                  xpool.tile([128, NDB // 2, QT], BF16, tag="xtB", name="xtB")]
            for hlf in range(2):
                nc.sync.dma_start(
                    out=xt[hlf],
                    in_=bass.AP(tensor=xd[:].tensor,
                                offset=st * QT + hlf * (NDB // 2) * 128 * S,
                                ap=[[S, 128], [128 * S, NDB // 2], [1, QT]]))
            psE = ps_av.tile([128, QT], F32, tag="av", name="pE")
            psO = ps_av.tile([128, QT], F32, tag="av", name="pO")

            def pair(db):
                def go():
                    for mt, ps in ((0, psE), (1, psO)):
                        nc.tensor.matmul(
                            ps[:, :],
                            lhsT=ws[:, db, mt * 128:(mt + 1) * 128],
                            rhs=xt[db // 4][:, db % 4, :],
                            start=(db == 0), stop=(db == NDB - 1),
                        )
                return go

            def finish():
                sl = slice(st * QT, (st + 1) * QT)
                t1 = ropet.tile([128, QT], F32, tag="t1")
                t2 = ropet.tile([128, QT], F32, tag="t2")
                t3 = ropet.tile([128, QT], F32, tag="t3")
                t4 = ropet.tile([128, QT], F32, tag="t4")
                nc.vector.scalar_tensor_tensor(
                    out=t1, in0=psE, scalar=bias_sb[:, 0:1],
                    in1=sin4[:, sl], op0=OP.add, op1=OP.mult)
                nc.vector.scalar_tensor_tensor(
                    out=t3, in0=psE, scalar=bias_sb[:, 0:1],
                    in1=cos4[:, sl], op0=OP.add, op1=OP.mult)
                nc.vector.scalar_tensor_tensor(
                    out=t2, in0=psO, scalar=bias_sb[:, 1:2],
                    in1=cos4[:, sl], op0=OP.add, op1=OP.mult)
                nc.vector.scalar_tensor_tensor(
                    out=t4, in0=psO, scalar=bias_sb[:, 1:2],
                    in1=sin4[:, sl], op0=OP.add, op1=OP.mult)
                eo = ropet.tile([128, 2, QT], BF16, tag="eo")
                nc.vector.tensor_sub(eo[:, 0, :], t3, t4)    # evens'
                nc.vector.tensor_add(eo[:, 1, :], t1, t2)    # odds'
                # one DMA per dst tile: the element stream (partition-major,
                # then [eo, q]) lands as rows [e0,o0,e1,o1,...] per head --
                # natural interleaved head-contiguous layout, same for Q & K.
                for t in range(2):
                    nc.scalar.dma_start(
                        out=dst[t][:, sl],
                        in_=eo[64 * t:64 * t + 64, :, :])
            return [pair(db) for db in range(NDB)], finish

        def proj_v(st):
            # V rows [s in st*QT..(st+1)*QT) -> kblocks 4st..4st+4 (ps_big)
            for half in range(2):
                s0 = st * QT + half * 256
                xv = xpool.tile([128, NDB, 256], BF16, tag="xv")
                nc.sync.dma_start(
                    out=xv,
                    in_=bass.AP(tensor=xvT_d[:].tensor, offset=s0,
                                ap=[[S, 128], [128 * S, NDB], [1, 256]]))
                ps2 = ps_big.tile([128, 2, QT], F32, tag="big", name="pv")
                ps = [ps2[:, 0, 0:DH], ps2[:, 1, 0:DH]]
                for db in range(NDB):
                    for kbl in range(2):
                        nc.tensor.matmul(
                            ps[kbl][:, :],
                            lhsT=xv[:, db, kbl * 128:(kbl + 1) * 128],
                            rhs=wv[:, db, :],
                            start=(db == 0), stop=(db == NDB - 1),
                        )
                for kbl in range(2):
                    kb = s0 // 128 + kbl
                    nc.vector.tensor_add(
                        v_sb[:, kb, :, 0:DK],
                        ps[kbl].rearrange("p (h d) -> p h d", h=HPC),
                        bv_bc.rearrange("p (h d) -> p h d", h=HPC),
                    )

        def normalize_finish(qt, avs_list, rec4):
            # bc matmuls + otmp muls + oT DMA for all 4 heads of qt; called
            # one stage later so the reciprocal (DVE) has long finished.
            for h in range(HPC):
                t, hh = h // 2, h % 2
                avs = avs_list[h]
                bc_ps = ps_av.tile([64, QT], F32, tag="av", name="bc")
                nc.tensor.matmul(bc_ps[:, :], lhsT=eye4[:, h, :],
                                 rhs=rec4[:, :], start=True, stop=True)
                bc_sb = opool.tile([64, QT], F32, tag="bc")
                nc.scalar.activation(out=bc_sb, in_=bc_ps, func=AF.Copy)
                otmp = opool.tile([64, QT], BF16, tag="otmp")
                nc.vector.tensor_mul(otmp, avs[0:DK, :], bc_sb)
                nc.gpsimd.dma_start(
                    out=oT[t][64 * hh:64 * hh + 64, qt * QT:(qt + 1) * QT],
                    in_=otmp)

        def outproj_jb(qt, jb, pool, tag):
            po = pool.tile([128, QT], F32, tag=tag, name="po") if tag == "av" \
                else pool.tile([128, 2, QT], F32, tag=tag, name="po")[:, 0, :]
            for mb in range(2):
                nc.tensor.matmul(
                    po[:, :],
                    lhsT=woT_sb[:, mb, jb * 128:(jb + 1) * 128],
                    rhs=oT[mb][:, qt * QT:(qt + 1) * QT],
                    start=(mb == 0), stop=(mb == 1),
                )
            ob = opool.tile([128, QT], F32, tag="ob")
            nc.vector.tensor_copy(ob, po)
            nc.sync.dma_start(
                out=out_d[jb * 128:(jb + 1) * 128, qt * QT:(qt + 1) * QT],
                in_=ob)

        def outproj(qt):
            for jb in range(8):
                outproj_jb(qt, jb, ps_big, "big")

        def attention_hp(qt, hp, av2, filler):
            nkb = 4 * qt + 4 if causal else NKB
            pending_av = []   # [(kb, c0, pt2)] awaiting AV matmuls

            def flush_av(last, depth=0):
                while len(pending_av) > depth:
                    pkb, pc0, ppt2 = pending_av.pop(0)
                    for j in range(2):
                        h = 2 * hp + j
                        nc.tensor.matmul(
                            av2[j][:, pc0:QT],
                            lhsT=v_sb[:, pkb, h, :],
                            rhs=ppt2[:, j, pc0:QT],
                            start=(pkb == 0),
                            stop=(last and not pending_av),
                        )

            nfil = len(filler)
            nspread = max(1, (nkb * 4) // 5)   # finish fillers by ~80% in
            for kb in range(nkb):
                lo = min(kb, nspread) * nfil // nspread
                hi = min(kb + 1, nspread) * nfil // nspread
                for f in filler[lo:hi]:
                    f()
                diag = causal and (kb >= 4 * qt)
                c0 = 128 * (kb - 4 * qt) if diag else 0
                sc2 = ps_big.tile([128, 2, QT], F32, tag="big", name="sc2")
                if not causal:
                    am = ampool.tile([KB, QT], F32, tag="am")
                    nc.sync.dma_start(
                        out=am,
                        in_=pat_d[kb * KB:(kb + 1) * KB,
                                  qt * QT:(qt + 1) * QT])
                for j in range(2):
                    h = 2 * hp + j
                    t, hh = h // 2, h % 2
                    nc.tensor.matmul(
                        sc2[:, j, c0:QT],
                        lhsT=kb_t[t][64 * hh:64 * hh + 64,
                                     kb * KB:(kb + 1) * KB],
                        rhs=qb[t][64 * hh:64 * hh + 64,
                                  qt * QT + c0:(qt + 1) * QT],
                        start=True, stop=True,
                    )
                    if causal and diag:
                        # mask the triangular [128,128] sub-block in place
                        nc.vector.scalar_tensor_tensor(
                            out=sc2[:, j, c0:c0 + KB],
                            in0=sc2[:, j, c0:c0 + KB],
                            scalar=0.0, in1=addtri,
                            op0=OP.add, op1=OP.add)
                    elif not causal:
                        nc.vector.scalar_tensor_tensor(
                            out=sc2[:, j, :], in0=sc2[:, j, :],
                            scalar=0.0, in1=am,
                            op0=OP.add, op1=OP.add)
                pt2 = ppool.tile([128, 2, QT], BF16, tag="pt")
                nc.scalar.activation(
                    out=pt2[:, :, c0:QT], in_=sc2[:, :, c0:QT],
                    func=AF.Exp, scale=0.125)
                pending_av.append((kb, c0, pt2))
                flush_av(last=False, depth=3)
            flush_av(last=True)

        # ---- emission: prologue proj(0), then per qt the attention stream
        # with next-tile projection matmuls interleaved as PE filler ----
        nc.sync.dma_start(out=wq, in_=wqT_d.rearrange("(n p) m -> p n m", p=128))
        pairs, fin = make_qkproj(xqT_d, wq, bq_sb, qb, 0)
        nc.sync.dma_start(out=wk, in_=wkT_d.rearrange("(n p) m -> p n m", p=128))
        for p in pairs:
            p()
        fin()
        pairs, fin = make_qkproj(xkT_d, wk, bk_sb, kb_t, 0)
        nc.sync.dma_start(out=wv, in_=wvT_d.rearrange("(n p) m -> p n m", p=128))
        for p in pairs:
            p()
        fin()
        proj_v(0)

        norm_state = {}
        for st in range(NQT):
            if st >= 1:
                normalize_finish(st - 1, *norm_state[st - 1])
            avs_list = []
            for hp in range(2):
                if st < NQT - 1:
                    if hp == 0:
                        filler, fin = make_qkproj(xqT_d, wq, bq_sb, qb, st + 1)
                    else:
                        filler, fin = make_qkproj(xkT_d, wk, bk_sb, kb_t, st + 1)
                    filler = filler + [fin]
                    fin = None
                else:
                    # last tile: earlier tiles' output projections are the
                    # PE filler (their oT finished stages ago)
                    pqt = NQT - 3 if hp == 0 else NQT - 2
                    filler = [(lambda jb=jb, pqt=pqt:
                               outproj_jb(pqt, jb, ps_av, "av"))
                              for jb in range(8)]
                    fin = None
                av2 = [ps_av.tile([DK + 1, QT], F32, tag="av", name=f"av{j}")
                       for j in range(2)]
                attention_hp(st, hp, av2, filler)
                for j in range(2):
                    avs = opool.tile([65, QT], F32, tag="avs", bufs=5)
                    nc.vector.tensor_copy(avs, av2[j])
                    avs_list.append(avs)
                if fin is not None:
                    fin()
                if hp == 0 and st == 1:
                    outproj(st - 1)
            lrow4 = opool.tile([4, QT], F32, tag="lrow4", bufs=2)
            for h in range(HPC):
                nc.gpsimd.dma_start(out=lrow4[h:h + 1, :],
                                    in_=avs_list[h][64:65, :])
            rec4 = opool.tile([4, QT], F32, tag="rec4", bufs=2)
            nc.vector.reciprocal(rec4, lrow4)
            norm_state[st] = (avs_list, rec4)
            if st < NQT - 1:
                proj_v(st + 1)
        normalize_finish(NQT - 1, *norm_state[NQT - 1])
        outproj(NQT - 1)

    with tile.TileContext(nc) as tc:
        emit(tc)
    nc.compile()
    return nc


def _host_prep(inputs):
    k, q, v = inputs["k"], inputs["q"], inputs["v"]
    mask, sin, cos = inputs["mask"], inputs["sin"], inputs["cos"]
    Wq, bq = inputs["Wq"], inputs["bq"]
    Wk, bk = inputs["Wk"], inputs["bk"]
    Wv, bv = inputs["Wv"], inputs["bv"]
    Wo = inputs["Wo"]

    causal = bool(np.array_equal(
        np.asarray(mask[0, 0]),
        np.tril(np.ones((S, S), np.asarray(mask).dtype))))

    sinT = np.ascontiguousarray(np.asarray(sin[0, 0]).T.astype(np.float32))
    cosT = np.ascontiguousarray(np.asarray(cos[0, 0]).T.astype(np.float32))
    sin4 = np.ascontiguousarray(np.tile(sinT, (4, 1)))
    cos4 = np.ascontiguousarray(np.tile(cosT, (4, 1)))

    if causal:
        ki = np.arange(KB)[:, None]
        qi = np.arange(KB)[None, :]
        pat = np.ascontiguousarray(
            np.where(ki <= qi, 0.0, NEG8).astype(np.float32))
        amaskT = None
    else:
        pat = None
        amaskT = np.ascontiguousarray(
            np.where(np.asarray(mask[0, 0]).T == 0, NEG8, 0.0).astype(np.float32))

    # E/O permutation of the 256 local head dims:
    # [all heads' even dims | all heads' odd dims]
    ev, od = np.arange(0, DK, 2), np.arange(1, DK, 2)
    perm = np.concatenate(
        [64 * h + ev for h in range(4)] + [64 * h + od for h in range(4)])

    eye4 = np.ascontiguousarray(
        np.eye(4, dtype=np.float32)[:, :, None].repeat(64, axis=2))

    xT = {}
    for name, x in (("q", q), ("k", k), ("v", v)):
        for b in range(B):
            xT[(name, b)] = np.ascontiguousarray(
                np.asarray(x[b]).T.astype(BF))

    in_maps = []
    for c in range(NCORES):
        b, hg = c // 4, c % 4
        rows = slice(hg * DH, (hg + 1) * DH)
        wqT = np.ascontiguousarray(np.asarray(Wq)[rows, :].T[:, perm])
        wkT = np.ascontiguousarray(np.asarray(Wk)[rows, :].T[:, perm])
        wvT = np.ascontiguousarray(np.asarray(Wv)[rows, :].T)
        woT = np.ascontiguousarray(np.asarray(Wo)[:, rows].T)
        m = dict(
            xqT=xT[("q", b)], xkT=xT[("k", b)], xvT=xT[("v", b)],
            wqT=wqT.astype(BF), wkT=wkT.astype(BF),
            wvT=wvT.astype(BF), woT=woT.astype(BF),
            bqA=np.ascontiguousarray(np.asarray(bq)[rows][perm]).astype(np.float32),
            bkA=np.ascontiguousarray(np.asarray(bk)[rows][perm]).astype(np.float32),
            bv=np.ascontiguousarray(np.asarray(bv)[rows]).astype(np.float32),
            sin4=sin4, cos4=cos4, eye4=eye4,
        )
        if causal:
            m["addtri"] = pat
        else:
            m["amaskT"] = amaskT
        in_maps.append(m)
    return causal, in_maps


def kernel(**inputs):
    from concourse.bass_utils import run_bass_kernel_spmd

    causal, in_maps = _host_prep(inputs)
    if causal not in _cache:
        _cache[causal] = _build_nc(causal)
    nc = _cache[causal]

    res = run_bass_kernel_spmd(nc, in_maps, core_ids=list(range(NCORES))).results

    bo = np.asarray(inputs["bo"]).astype(np.float32)
    out = np.empty((B, S, D), np.float32)
    for b in range(B):
        acc = res[4 * b]["outT"].astype(np.float32).copy()
        for c in range(4 * b + 1, 4 * b + 4):
            acc += res[c]["outT"]
        out[b] = acc.T + bo
    return out


# revision 29
# speedup vs baseline: 1.0441x; 1.0441x over previous
"""Multi-head attention (B=2, S=2048, D=1024, H=16, DK=64) with RoPE and
causal masking, sharded over 8 NeuronCores as (batch x head-group):
core c handles batch c//4 and heads 4*(c%4) .. 4*(c%4)+4.

Per-core dataflow (all layouts chosen so no on-device transposes are needed):
  - host pre-transposes activations X^T [D, S] and weight slices.
  - projections produce Q^T/K^T in an "A layout" per 128-partition tile
    ([h0e|h1e|h0o|h1o]: RoPE even/odd dims grouped) via PE matmul,
    evicted from PSUM with fused bias add (DVE tensor_scalar_add).
  - RoPE runs full-width DVE ops on the A tiles and re-packs into the
    "B layout" ([h0e'|h0o'|h1e'|h1o']: head-contiguous, 64 dims/head).
  - scores_t[k,q] = K_B.T @ Q_B per (qtile 512, kblock 128, head), K_c=64.
    Softmax without max-subtraction (scores ~ N(0,1), safe in fp32):
    P = exp(0.125*s + addpat) where addpat is 0/-1e30; causal blocks above
    the diagonal are skipped entirely.
  - AV: lhsT = [V | ones] (M=65) stationary, rhs = P_t moving; PSUM
    accumulates over kblocks; row 64 gives the softmax denominator l.
  - normalize: O^T = AV[0:64] * broadcast(1/l) (broadcast via K_c=1 matmul).
  - output projection: partial^T[j,s] = woT.T @ O^T; host sums the 4
    partials per batch, transposes, and adds bo.

The whole thing is emitted as one software pipeline over the 4 q s-tiles:
projections(st) -> RoPE(st) -> attention(qt=st) -> out-proj(st), so Tile
overlaps DMA/PE/DVE/ACT across phases.
"""
import numpy as np
import ml_dtypes

BF = ml_dtypes.bfloat16
B, S, D, H, DK = 2, 2048, 1024, 16, 64
NCORES = 8
HPC = 4            # heads per core
DH = HPC * DK      # 256 local head dims
QT = 512           # q tile (free dim of scores matmul)
KB = 128           # k block (partition dim of scores)
NQT = S // QT      # 4
NKB = S // KB      # 16
NDB = D // 128     # 8 d-blocks for projections
NEG = -1.0e30
NEG8 = -1.0e31   # pre-scale mask value: *0.125 -> -1.25e30

_cache = {}


def _build_nc(causal: bool):
    from contextlib import ExitStack
    import concourse.bass as bass
    import concourse.tile as tile
    from concourse import bacc, mybir
    from concourse._compat import with_exitstack

    F32 = mybir.dt.float32
    BF16 = mybir.dt.bfloat16
    AF = mybir.ActivationFunctionType
    OP = mybir.AluOpType

    nc = bacc.Bacc(None, target_bir_lowering=False, debug=False)

    xqT_d = nc.dram_tensor("xqT", [D, S], BF16, kind="ExternalInput")
    xkT_d = nc.dram_tensor("xkT", [D, S], BF16, kind="ExternalInput")
    xvT_d = nc.dram_tensor("xvT", [D, S], BF16, kind="ExternalInput")
    wqT_d = nc.dram_tensor("wqT", [D, DH], BF16, kind="ExternalInput")
    wkT_d = nc.dram_tensor("wkT", [D, DH], BF16, kind="ExternalInput")
    wvT_d = nc.dram_tensor("wvT", [D, DH], BF16, kind="ExternalInput")
    bqA_d = nc.dram_tensor("bqA", [DH], F32, kind="ExternalInput")
    bkA_d = nc.dram_tensor("bkA", [DH], F32, kind="ExternalInput")
    bv_d = nc.dram_tensor("bv", [DH], F32, kind="ExternalInput")
    woT_d = nc.dram_tensor("woT", [DH, D], BF16, kind="ExternalInput")
    sin_d = nc.dram_tensor("sin4", [128, S], F32, kind="ExternalInput")
    cos_d = nc.dram_tensor("cos4", [128, S], F32, kind="ExternalInput")
    if causal:
        pat_d = nc.dram_tensor("addtri", [KB, KB], F32, kind="ExternalInput")
    else:
        pat_d = nc.dram_tensor("amaskT", [S, S], F32, kind="ExternalInput")
    eye4_d = nc.dram_tensor("eye4", [128, 4, 64], F32, kind="ExternalInput")
    out_d = nc.dram_tensor("outT", [D, S], F32, kind="ExternalOutput")

    @with_exitstack
    def emit(ctx: ExitStack, tc: tile.TileContext):
        nc = tc.nc
        singles = ctx.enter_context(tc.tile_pool(name="singles", bufs=1))
        xpool = ctx.enter_context(tc.tile_pool(name="x", bufs=3))
        ropet = ctx.enter_context(tc.tile_pool(name="ropet", bufs=2))
        bpool = ctx.enter_context(tc.tile_pool(name="bpool", bufs=1))
        ppool = ctx.enter_context(tc.tile_pool(name="ppool", bufs=8))
        opool = ctx.enter_context(tc.tile_pool(name="opool", bufs=3))
        if not causal:
            ampool = ctx.enter_context(tc.tile_pool(name="ampool", bufs=3))

        # "big" slots are 2 banks each ([128,2,QT]); proj psum pairs and
        # score groups rotate through them. av holds AV accumulators and the
        # tiny broadcast matmuls.
        ps_big = ctx.enter_context(tc.tile_pool(name="ps_big", bufs=2, space="PSUM"))
        ps_av = ctx.enter_context(tc.tile_pool(name="ps_av", bufs=4, space="PSUM"))

        # ---- constants on the gpsimd queue; weights go on sync, each
        # emitted right before its first consumer ----
        wq = singles.tile([128, NDB, DH], BF16)
        wk = singles.tile([128, NDB, DH], BF16)
        wv = singles.tile([128, NDB, DH], BF16)
        sin4 = singles.tile([128, S], F32)
        cos4 = singles.tile([128, S], F32)
        nc.gpsimd.dma_start(out=sin4, in_=sin_d[:])
        nc.gpsimd.dma_start(out=cos4, in_=cos_d[:])
        eye4 = singles.tile([128, 4, 64], F32)
        nc.gpsimd.dma_start(out=eye4, in_=eye4_d[:])
        if causal:
            addtri = singles.tile([KB, KB], F32)
            nc.gpsimd.dma_start(out=addtri, in_=pat_d[:])

        bq_sb = singles.tile([128, 2], F32)
        bk_sb = singles.tile([128, 2], F32)
        nc.gpsimd.dma_start(out=bq_sb, in_=bqA_d.rearrange("(t p) -> p t", p=128))
        nc.gpsimd.dma_start(out=bk_sb, in_=bkA_d.rearrange("(t p) -> p t", p=128))
        bv_bc = singles.tile([128, DH], F32)
        nc.gpsimd.dma_start(
            out=bv_bc,
            in_=bass.AP(tensor=bv_d[:].tensor, offset=0, ap=[[0, 128], [1, DH]]),
        )

        woT_sb = singles.tile([128, 2, D], BF16)
        nc.gpsimd.dma_start(
            out=woT_sb, in_=woT_d.rearrange("(n p) m -> p n m", p=128))

        # persistent activations (post-RoPE Q/K and attention output, T layout)
        qb = [bpool.tile([128, S], BF16, tag=f"qb{t}", name=f"qb{t}") for t in range(2)]
        kb_t = [bpool.tile([128, S], BF16, tag=f"kbt{t}", name=f"kbt{t}") for t in range(2)]
        oT = [bpool.tile([128, S], BF16, tag=f"oT{t}", name=f"oT{t}") for t in range(2)]
        v_sb = singles.tile([128, NKB, HPC, DK + 1], BF16)
        nc.vector.memset(v_sb[:, :, :, DK:DK + 1], 1.0)

        def make_qkproj(xd, ws, bias_sb, dst, st):
            """Returns (pairs, finish): `pairs` are per-d-block closures each
            emitting the E+O matmuls (interleaved into the attention stream
            as PE keep-alive filler); `finish` emits bias+RoPE (STT) and the
            stream-order scatter DMA into the head-contiguous dst tiles."""
            xt = [xpool.tile([128, NDB // 2, QT], BF16, tag="xtA", name="xtA"),
                  xpool.tile([128, NDB // 2, QT], BF16, tag="xtB", name="xtB")]
            for hlf in range(2):
                nc.sync.dma_start(
                    out=xt[hlf],
                    in_=bass.AP(tensor=xd[:].tensor,
                                offset=st * QT + hlf * (NDB // 2) * 128 * S,
                                ap=[[S, 128], [128 * S, NDB // 2], [1, QT]]))
            psE = ps_av.tile([128, QT], F32, tag="av", name="pE")
            psO = ps_av.tile([128, QT], F32, tag="av", name="pO")

            def pair(db):
                def go():
                    for mt, ps in ((0, psE), (1, psO)):
                        nc.tensor.matmul(
                            ps[:, :],
                            lhsT=ws[:, db, mt * 128:(mt + 1) * 128],
                            rhs=xt[db // 4][:, db % 4, :],
                            start=(db == 0), stop=(db == NDB - 1),
                        )
                return go

            def finish():
                sl = slice(st * QT, (st + 1) * QT)
                t1 = ropet.tile([128, QT], F32, tag="t1")
                t2 = ropet.tile([128, QT], F32, tag="t2")
                t3 = ropet.tile([128, QT], F32, tag="t3")
                t4 = ropet.tile([128, QT], F32, tag="t4")
                nc.vector.scalar_tensor_tensor(
                    out=t1, in0=psE, scalar=bias_sb[:, 0:1],
                    in1=sin4[:, sl], op0=OP.add, op1=OP.mult)
                nc.vector.scalar_tensor_tensor(
                    out=t3, in0=psE, scalar=bias_sb[:, 0:1],
                    in1=cos4[:, sl], op0=OP.add, op1=OP.mult)
                nc.vector.scalar_tensor_tensor(
                    out=t2, in0=psO, scalar=bias_sb[:, 1:2],
                    in1=cos4[:, sl], op0=OP.add, op1=OP.mult)
                nc.vector.scalar_tensor_tensor(
                    out=t4, in0=psO, scalar=bias_sb[:, 1:2],
                    in1=sin4[:, sl], op0=OP.add, op1=OP.mult)
                eo = ropet.tile([128, 2, QT], BF16, tag="eo")
                nc.vector.tensor_sub(eo[:, 0, :], t3, t4)    # evens'
                nc.vector.tensor_add(eo[:, 1, :], t1, t2)    # odds'
                # one DMA per dst tile: the element stream (partition-major,
                # then [eo, q]) lands as rows [e0,o0,e1,o1,...] per head --
                # natural interleaved head-contiguous layout, same for Q & K.
                for t in range(2):
                    nc.scalar.dma_start(
                        out=dst[t][:, sl],
                        in_=eo[64 * t:64 * t + 64, :, :])
            return [pair(db) for db in range(NDB)], finish

        def proj_v(st):
            # V rows [s in st*QT..(st+1)*QT) -> kblocks 4st..4st+4 (ps_big)
            for half in range(2):
                s0 = st * QT + half * 256
                xv = xpool.tile([128, NDB, 256], BF16, tag="xv")
                nc.sync.dma_start(
                    out=xv,
                    in_=bass.AP(tensor=xvT_d[:].tensor, offset=s0,
                                ap=[[S, 128], [128 * S, NDB], [1, 256]]))
                ps2 = ps_big.tile([128, 2, QT], F32, tag="big", name="pv")
                ps = [ps2[:, 0, 0:DH], ps2[:, 1, 0:DH]]
                for db in range(NDB):
                    for kbl in range(2):
                        nc.tensor.matmul(
                            ps[kbl][:, :],
                            lhsT=xv[:, db, kbl * 128:(kbl + 1) * 128],
                            rhs=wv[:, db, :],
                            start=(db == 0), stop=(db == NDB - 1),
                        )
                for kbl in range(2):
                    kb = s0 // 128 + kbl
                    nc.vector.tensor_add(
                        v_sb[:, kb, :, 0:DK],
                        ps[kbl].rearrange("p (h d) -> p h d", h=HPC),
                        bv_bc.rearrange("p (h d) -> p h d", h=HPC),
                    )

        def normalize_finish(qt, avs_list, rec4):
            # bc matmuls + otmp muls + oT DMA for all 4 heads of qt; called
            # one stage later so the reciprocal (DVE) has long finished.
            for h in range(HPC):
                t, hh = h // 2, h % 2
                avs = avs_list[h]
                bc_ps = ps_av.tile([64, QT], F32, tag="av", name="bc")
                nc.tensor.matmul(bc_ps[:, :], lhsT=eye4[:, h, :],
                                 rhs=rec4[:, :], start=True, stop=True)
                bc_sb = opool.tile([64, QT], F32, tag="bc")
                nc.scalar.activation(out=bc_sb, in_=bc_ps, func=AF.Copy)
                otmp = opool.tile([64, QT], BF16, tag="otmp")
                nc.vector.tensor_mul(otmp, avs[0:DK, :], bc_sb)
                nc.gpsimd.dma_start(
                    out=oT[t][64 * hh:64 * hh + 64, qt * QT:(qt + 1) * QT],
                    in_=otmp)

        def outproj_jb(qt, jb, pool, tag):
            po = pool.tile([128, QT], F32, tag=tag, name="po") if tag == "av" \
                else pool.tile([128, 2, QT], F32, tag=tag, name="po")[:, 0, :]
            for mb in range(2):
                nc.tensor.matmul(
                    po[:, :],
                    lhsT=woT_sb[:, mb, jb * 128:(jb + 1) * 128],
                    rhs=oT[mb][:, qt * QT:(qt + 1) * QT],
                    start=(mb == 0), stop=(mb == 1),
                )
            ob = opool.tile([128, QT], F32, tag="ob")
            nc.vector.tensor_copy(ob, po)
            nc.sync.dma_start(
                out=out_d[jb * 128:(jb + 1) * 128, qt * QT:(qt + 1) * QT],
                in_=ob)

        def outproj(qt):
            for jb in range(8):
                outproj_jb(qt, jb, ps_big, "big")

        def attention_hp(qt, hp, av2, filler):
            nkb = 4 * qt + 4 if causal else NKB
            pending_av = []   # [(kb, c0, pt2)] awaiting AV matmuls

            def flush_av(last, depth=0):
                while len(pending_av) > depth:
                    pkb, pc0, ppt2 = pending_av.pop(0)
                    for j in range(2):
                        h = 2 * hp + j
                        nc.tensor.matmul(
                            av2[j][:, pc0:QT],
                            lhsT=v_sb[:, pkb, h, :],
                            rhs=ppt2[:, j, pc0:QT],
                            start=(pkb == 0),
                            stop=(last and not pending_av),
                        )

            nfil = len(filler)
            nspread = max(1, (nkb * 4) // 5)   # finish fillers by ~80% in
            for kb in range(nkb):
                lo = min(kb, nspread) * nfil // nspread
                hi = min(kb + 1, nspread) * nfil // nspread
                for f in filler[lo:hi]:
                    f()
                diag = causal and (kb >= 4 * qt)
                c0 = 128 * (kb - 4 * qt) if diag else 0
                sc2 = ps_big.tile([128, 2, QT], F32, tag="big", name="sc2")
                if not causal:
                    am = ampool.tile([KB, QT], F32, tag="am")
                    nc.sync.dma_start(
                        out=am,
                        in_=pat_d[kb * KB:(kb + 1) * KB,
                                  qt * QT:(qt + 1) * QT])
                for j in range(2):
                    h = 2 * hp + j
                    t, hh = h // 2, h % 2
                    nc.tensor.matmul(
                        sc2[:, j, c0:QT],
                        lhsT=kb_t[t][64 * hh:64 * hh + 64,
                                     kb * KB:(kb + 1) * KB],
                        rhs=qb[t][64 * hh:64 * hh + 64,
                                  qt * QT + c0:(qt + 1) * QT],
                        start=True, stop=True,
                    )
                    if causal and diag:
                        # mask the triangular [128,128] sub-block in place
                        nc.vector.scalar_tensor_tensor(
                            out=sc2[:, j, c0:c0 + KB],
                            in0=sc2[:, j, c0:c0 + KB],
                            scalar=0.0, in1=addtri,
                            op0=OP.add, op1=OP.add)
                    elif not causal:
                        nc.vector.scalar_tensor_tensor(
                            out=sc2[:, j, :], in0=sc2[:, j, :],
                            scalar=0.0, in1=am,
                            op0=OP.add, op1=OP.add)
                pt2 = ppool.tile([128, 2, QT], BF16, tag="pt")
                nc.scalar.activation(
                    out=pt2[:, :, c0:QT], in_=sc2[:, :, c0:QT],
                    func=AF.Exp, scale=0.125)
                pending_av.append((kb, c0, pt2))
                flush_av(last=False, depth=3)
            flush_av(last=True)

        # ---- emission: prologue proj(0), then per qt the attention stream
        # with next-tile projection matmuls interleaved as PE filler ----
        nc.sync.dma_start(out=wq, in_=wqT_d.rearrange("(n p) m -> p n m", p=128))
        pairs, fin = make_qkproj(xqT_d, wq, bq_sb, qb, 0)
        nc.sync.dma_start(out=wk, in_=wkT_d.rearrange("(n p) m -> p n m", p=128))
        for p in pairs:
            p()
        fin()
        pairs, fin = make_qkproj(xkT_d, wk, bk_sb, kb_t, 0)
        nc.sync.dma_start(out=wv, in_=wvT_d.rearrange("(n p) m -> p n m", p=128))
        for p in pairs:
            p()
        fin()
        proj_v(0)

        norm_state = {}
        for st in range(NQT):
            if st >= 1:
                normalize_finish(st - 1, *norm_state[st - 1])
            avs_list = []
            for hp in range(2):
                if st < NQT - 1:
                    if hp == 0:
                        filler, fin = make_qkproj(xqT_d, wq, bq_sb, qb, st + 1)
                    else:
                        filler, fin = make_qkproj(xkT_d, wk, bk_sb, kb_t, st + 1)
                    filler = filler + [fin]
                    fin = None
                else:
                    # last tile: earlier tiles' output projections are the
                    # PE filler (their oT finished stages ago)
                    pqt = NQT - 3 if hp == 0 else NQT - 2
                    filler = [(lambda jb=jb, pqt=pqt:
                               outproj_jb(pqt, jb, ps_av, "av"))
                              for jb in range(8)]
                    fin = None
                av2 = [ps_av.tile([DK + 1, QT], F32, tag="av", name=f"av{j}")
                       for j in range(2)]
                attention_hp(st, hp, av2, filler)
                for j in range(2):
                    avs = opool.tile([65, QT], F32, tag="avs", bufs=5)
                    nc.vector.tensor_copy(avs, av2[j])
                    avs_list.append(avs)
                if fin is not None:
                    fin()
                if hp == 0 and st == 1:
                    outproj(st - 1)
            lrow4 = opool.tile([4, QT], F32, tag="lrow4", bufs=2)
            for h in range(HPC):
                nc.gpsimd.dma_start(out=lrow4[h:h + 1, :],
                                    in_=avs_list[h][64:65, :])
            # rec4 padded to 128 partitions (zeros) so the bc matmul reads
            # full SBUF bandwidth (K_c=4 was ~4x slower than K_c=128)
            rec4 = opool.tile([128, QT], F32, tag="rec4", bufs=2)
            nc.gpsimd.memset(rec4, 0.0)
            nc.vector.reciprocal(rec4[0:4, :], lrow4)
            norm_state[st] = (avs_list, rec4)
            if st < NQT - 1:
                proj_v(st + 1)
        normalize_finish(NQT - 1, *norm_state[NQT - 1])
        outproj(NQT - 1)

    with tile.TileContext(nc) as tc:
        emit(tc)
    nc.compile()
    return nc


def _host_prep(inputs):
    k, q, v = inputs["k"], inputs["q"], inputs["v"]
    mask, sin, cos = inputs["mask"], inputs["sin"], inputs["cos"]
    Wq, bq = inputs["Wq"], inputs["bq"]
    Wk, bk = inputs["Wk"], inputs["bk"]
    Wv, bv = inputs["Wv"], inputs["bv"]
    Wo = inputs["Wo"]

    causal = bool(np.array_equal(
        np.asarray(mask[0, 0]),
        np.tril(np.ones((S, S), np.asarray(mask).dtype))))

    sinT = np.ascontiguousarray(np.asarray(sin[0, 0]).T.astype(np.float32))
    cosT = np.ascontiguousarray(np.asarray(cos[0, 0]).T.astype(np.float32))
    sin4 = np.ascontiguousarray(np.tile(sinT, (4, 1)))
    cos4 = np.ascontiguousarray(np.tile(cosT, (4, 1)))

    if causal:
        ki = np.arange(KB)[:, None]
        qi = np.arange(KB)[None, :]
        pat = np.ascontiguousarray(
            np.where(ki <= qi, 0.0, NEG8).astype(np.float32))
        amaskT = None
    else:
        pat = None
        amaskT = np.ascontiguousarray(
            np.where(np.asarray(mask[0, 0]).T == 0, NEG8, 0.0).astype(np.float32))

    # E/O permutation of the 256 local head dims:
    # [all heads' even dims | all heads' odd dims]
    ev, od = np.arange(0, DK, 2), np.arange(1, DK, 2)
    perm = np.concatenate(
        [64 * h + ev for h in range(4)] + [64 * h + od for h in range(4)])

    eye4 = np.zeros((128, 4, 64), np.float32)
    for h in range(4):
        eye4[h, h, :] = 1.0

    xT = {}
    for name, x in (("q", q), ("k", k), ("v", v)):
        for b in range(B):
            xT[(name, b)] = np.ascontiguousarray(
                np.asarray(x[b]).T.astype(BF))

    in_maps = []
    for c in range(NCORES):
        b, hg = c // 4, c % 4
        rows = slice(hg * DH, (hg + 1) * DH)
        wqT = np.ascontiguousarray(np.asarray(Wq)[rows, :].T[:, perm])
        wkT = np.ascontiguousarray(np.asarray(Wk)[rows, :].T[:, perm])
        wvT = np.ascontiguousarray(np.asarray(Wv)[rows, :].T)
        woT = np.ascontiguousarray(np.asarray(Wo)[:, rows].T)
        m = dict(
            xqT=xT[("q", b)], xkT=xT[("k", b)], xvT=xT[("v", b)],
            wqT=wqT.astype(BF), wkT=wkT.astype(BF),
            wvT=wvT.astype(BF), woT=woT.astype(BF),
            bqA=np.ascontiguousarray(np.asarray(bq)[rows][perm]).astype(np.float32),
            bkA=np.ascontiguousarray(np.asarray(bk)[rows][perm]).astype(np.float32),
            bv=np.ascontiguousarray(np.asarray(bv)[rows]).astype(np.float32),
            sin4=sin4, cos4=cos4, eye4=eye4,
        )
        if causal:
            m["addtri"] = pat
        else:
            m["amaskT"] = amaskT
        in_maps.append(m)
    return causal, in_maps


def kernel(**inputs):
    from concourse.bass_utils import run_bass_kernel_spmd

    causal, in_maps = _host_prep(inputs)
    if causal not in _cache:
        _cache[causal] = _build_nc(causal)
    nc = _cache[causal]

    res = run_bass_kernel_spmd(nc, in_maps, core_ids=list(range(NCORES))).results

    bo = np.asarray(inputs["bo"]).astype(np.float32)
    out = np.empty((B, S, D), np.float32)
    for b in range(B):
        acc = res[4 * b]["outT"].astype(np.float32).copy()
        for c in range(4 * b + 1, 4 * b + 4):
            acc += res[c]["outT"]
        out[b] = acc.T + bo
    return out


# revision 30
# speedup vs baseline: 1.0554x; 1.0108x over previous
"""Multi-head attention (B=2, S=2048, D=1024, H=16, DK=64) with RoPE and
causal masking, sharded over 8 NeuronCores as (batch x head-group):
core c handles batch c//4 and heads 4*(c%4) .. 4*(c%4)+4.

Per-core dataflow (all layouts chosen so no on-device transposes are needed):
  - host pre-transposes activations X^T [D, S] and weight slices.
  - projections produce Q^T/K^T in an "A layout" per 128-partition tile
    ([h0e|h1e|h0o|h1o]: RoPE even/odd dims grouped) via PE matmul,
    evicted from PSUM with fused bias add (DVE tensor_scalar_add).
  - RoPE runs full-width DVE ops on the A tiles and re-packs into the
    "B layout" ([h0e'|h0o'|h1e'|h1o']: head-contiguous, 64 dims/head).
  - scores_t[k,q] = K_B.T @ Q_B per (qtile 512, kblock 128, head), K_c=64.
    Softmax without max-subtraction (scores ~ N(0,1), safe in fp32):
    P = exp(0.125*s + addpat) where addpat is 0/-1e30; causal blocks above
    the diagonal are skipped entirely.
  - AV: lhsT = [V | ones] (M=65) stationary, rhs = P_t moving; PSUM
    accumulates over kblocks; row 64 gives the softmax denominator l.
  - normalize: O^T = AV[0:64] * broadcast(1/l) (broadcast via K_c=1 matmul).
  - output projection: partial^T[j,s] = woT.T @ O^T; host sums the 4
    partials per batch, transposes, and adds bo.

The whole thing is emitted as one software pipeline over the 4 q s-tiles:
projections(st) -> RoPE(st) -> attention(qt=st) -> out-proj(st), so Tile
overlaps DMA/PE/DVE/ACT across phases.
"""
import numpy as np
import ml_dtypes

BF = ml_dtypes.bfloat16
B, S, D, H, DK = 2, 2048, 1024, 16, 64
NCORES = 8
HPC = 4            # heads per core
DH = HPC * DK      # 256 local head dims
QT = 512           # q tile (free dim of scores matmul)
KB = 128           # k block (partition dim of scores)
NQT = S // QT      # 4
NKB = S // KB      # 16
NDB = D // 128     # 8 d-blocks for projections
NEG = -1.0e30
NEG8 = -1.0e31   # pre-scale mask value: *0.125 -> -1.25e30

_cache = {}


def _build_nc(causal: bool):
    from contextlib import ExitStack
    import concourse.bass as bass
    import concourse.tile as tile
    from concourse import bacc, mybir
    from concourse._compat import with_exitstack

    F32 = mybir.dt.float32
    BF16 = mybir.dt.bfloat16
    AF = mybir.ActivationFunctionType
    OP = mybir.AluOpType

    nc = bacc.Bacc(None, target_bir_lowering=False, debug=False)

    xqT_d = nc.dram_tensor("xqT", [NQT, 128, NDB, QT], BF16, kind="ExternalInput")
    xkT_d = nc.dram_tensor("xkT", [NQT, 128, NDB, QT], BF16, kind="ExternalInput")
    xvT_d = nc.dram_tensor("xvT", [2 * NQT, 128, NDB, 256], BF16, kind="ExternalInput")
    wqT_d = nc.dram_tensor("wqT", [128, NDB, DH], BF16, kind="ExternalInput")
    wkT_d = nc.dram_tensor("wkT", [128, NDB, DH], BF16, kind="ExternalInput")
    wvT_d = nc.dram_tensor("wvT", [128, NDB, DH], BF16, kind="ExternalInput")
    bqA_d = nc.dram_tensor("bqA", [DH], F32, kind="ExternalInput")
    bkA_d = nc.dram_tensor("bkA", [DH], F32, kind="ExternalInput")
    bv_d = nc.dram_tensor("bv", [DH], F32, kind="ExternalInput")
    woT_d = nc.dram_tensor("woT", [128, 2, D], BF16, kind="ExternalInput")
    sin_d = nc.dram_tensor("sin4", [128, S], F32, kind="ExternalInput")
    cos_d = nc.dram_tensor("cos4", [128, S], F32, kind="ExternalInput")
    if causal:
        pat_d = nc.dram_tensor("addtri", [KB, KB], F32, kind="ExternalInput")
    else:
        pat_d = nc.dram_tensor("amaskT", [S, S], F32, kind="ExternalInput")
    eye4_d = nc.dram_tensor("eye4", [128, 4, 64], F32, kind="ExternalInput")
    out_d = nc.dram_tensor("outT", [D, S], F32, kind="ExternalOutput")

    @with_exitstack
    def emit(ctx: ExitStack, tc: tile.TileContext):
        nc = tc.nc
        singles = ctx.enter_context(tc.tile_pool(name="singles", bufs=1))
        xpool = ctx.enter_context(tc.tile_pool(name="x", bufs=3))
        ropet = ctx.enter_context(tc.tile_pool(name="ropet", bufs=2))
        bpool = ctx.enter_context(tc.tile_pool(name="bpool", bufs=1))
        ppool = ctx.enter_context(tc.tile_pool(name="ppool", bufs=8))
        opool = ctx.enter_context(tc.tile_pool(name="opool", bufs=3))
        if not causal:
            ampool = ctx.enter_context(tc.tile_pool(name="ampool", bufs=3))

        # "big" slots are 2 banks each ([128,2,QT]); proj psum pairs and
        # score groups rotate through them. av holds AV accumulators and the
        # tiny broadcast matmuls.
        ps_big = ctx.enter_context(tc.tile_pool(name="ps_big", bufs=2, space="PSUM"))
        ps_av = ctx.enter_context(tc.tile_pool(name="ps_av", bufs=4, space="PSUM"))

        # ---- constants on the gpsimd queue; weights go on sync, each
        # emitted right before its first consumer ----
        wq = singles.tile([128, NDB, DH], BF16)
        wk = singles.tile([128, NDB, DH], BF16)
        wv = singles.tile([128, NDB, DH], BF16)
        sin4 = singles.tile([128, S], F32)
        cos4 = singles.tile([128, S], F32)
        nc.gpsimd.dma_start(out=sin4, in_=sin_d[:])
        nc.gpsimd.dma_start(out=cos4, in_=cos_d[:])
        eye4 = singles.tile([128, 4, 64], F32)
        nc.gpsimd.dma_start(out=eye4, in_=eye4_d[:])
        if causal:
            addtri = singles.tile([KB, KB], F32)
            nc.gpsimd.dma_start(out=addtri, in_=pat_d[:])

        bq_sb = singles.tile([128, 2], F32)
        bk_sb = singles.tile([128, 2], F32)
        nc.gpsimd.dma_start(out=bq_sb, in_=bqA_d.rearrange("(t p) -> p t", p=128))
        nc.gpsimd.dma_start(out=bk_sb, in_=bkA_d.rearrange("(t p) -> p t", p=128))
        bv_bc = singles.tile([128, DH], F32)
        nc.gpsimd.dma_start(
            out=bv_bc,
            in_=bass.AP(tensor=bv_d[:].tensor, offset=0, ap=[[0, 128], [1, DH]]),
        )

        woT_sb = singles.tile([128, 2, D], BF16)
        nc.gpsimd.dma_start(out=woT_sb, in_=woT_d[:])

        # persistent activations (post-RoPE Q/K and attention output, T layout)
        qb = [bpool.tile([128, S], BF16, tag=f"qb{t}", name=f"qb{t}") for t in range(2)]
        kb_t = [bpool.tile([128, S], BF16, tag=f"kbt{t}", name=f"kbt{t}") for t in range(2)]
        oT = [bpool.tile([128, S], BF16, tag=f"oT{t}", name=f"oT{t}") for t in range(2)]
        v_sb = singles.tile([128, NKB, HPC, DK + 1], BF16)
        nc.vector.memset(v_sb[:, :, :, DK:DK + 1], 1.0)

        def make_qkproj(xd, ws, bias_sb, dst, st):
            """Returns (pairs, finish): `pairs` are per-d-block closures each
            emitting the E+O matmuls (interleaved into the attention stream
            as PE keep-alive filler); `finish` emits bias+RoPE (STT) and the
            stream-order scatter DMA into the head-contiguous dst tiles."""
            xt = [xpool.tile([128, NDB // 2, QT], BF16, tag="xtA", name="xtA"),
                  xpool.tile([128, NDB // 2, QT], BF16, tag="xtB", name="xtB")]
            for hlf in range(2):
                nc.sync.dma_start(
                    out=xt[hlf],
                    in_=xd[st, :, hlf * (NDB // 2):(hlf + 1) * (NDB // 2), :])
            psE = ps_av.tile([128, QT], F32, tag="av", name="pE")
            psO = ps_av.tile([128, QT], F32, tag="av", name="pO")

            def pair(db):
                def go():
                    for mt, ps in ((0, psE), (1, psO)):
                        nc.tensor.matmul(
                            ps[:, :],
                            lhsT=ws[:, db, mt * 128:(mt + 1) * 128],
                            rhs=xt[db // 4][:, db % 4, :],
                            start=(db == 0), stop=(db == NDB - 1),
                        )
                return go

            def finish():
                sl = slice(st * QT, (st + 1) * QT)
                t1 = ropet.tile([128, QT], F32, tag="t1")
                t2 = ropet.tile([128, QT], F32, tag="t2")
                t3 = ropet.tile([128, QT], F32, tag="t3")
                t4 = ropet.tile([128, QT], F32, tag="t4")
                nc.vector.scalar_tensor_tensor(
                    out=t1, in0=psE, scalar=bias_sb[:, 0:1],
                    in1=sin4[:, sl], op0=OP.add, op1=OP.mult)
                nc.vector.scalar_tensor_tensor(
                    out=t3, in0=psE, scalar=bias_sb[:, 0:1],
                    in1=cos4[:, sl], op0=OP.add, op1=OP.mult)
                nc.vector.scalar_tensor_tensor(
                    out=t2, in0=psO, scalar=bias_sb[:, 1:2],
                    in1=cos4[:, sl], op0=OP.add, op1=OP.mult)
                nc.vector.scalar_tensor_tensor(
                    out=t4, in0=psO, scalar=bias_sb[:, 1:2],
                    in1=sin4[:, sl], op0=OP.add, op1=OP.mult)
                eo = ropet.tile([128, 2, QT], BF16, tag="eo")
                nc.vector.tensor_sub(eo[:, 0, :], t3, t4)    # evens'
                nc.vector.tensor_add(eo[:, 1, :], t1, t2)    # odds'
                # one DMA per dst tile: the element stream (partition-major,
                # then [eo, q]) lands as rows [e0,o0,e1,o1,...] per head --
                # natural interleaved head-contiguous layout, same for Q & K.
                for t in range(2):
                    nc.scalar.dma_start(
                        out=dst[t][:, sl],
                        in_=eo[64 * t:64 * t + 64, :, :])
            return [pair(db) for db in range(NDB)], finish

        def proj_v(st):
            # V rows [s in st*QT..(st+1)*QT) -> kblocks 4st..4st+4 (ps_big)
            for half in range(2):
                s0 = st * QT + half * 256
                xv = xpool.tile([128, NDB, 256], BF16, tag="xv")
                nc.sync.dma_start(out=xv, in_=xvT_d[2 * st + half])
                ps2 = ps_big.tile([128, 2, QT], F32, tag="big", name="pv")
                ps = [ps2[:, 0, 0:DH], ps2[:, 1, 0:DH]]
                for db in range(NDB):
                    for kbl in range(2):
                        nc.tensor.matmul(
                            ps[kbl][:, :],
                            lhsT=xv[:, db, kbl * 128:(kbl + 1) * 128],
                            rhs=wv[:, db, :],
                            start=(db == 0), stop=(db == NDB - 1),
                        )
                for kbl in range(2):
                    kb = s0 // 128 + kbl
                    nc.vector.tensor_add(
                        v_sb[:, kb, :, 0:DK],
                        ps[kbl].rearrange("p (h d) -> p h d", h=HPC),
                        bv_bc.rearrange("p (h d) -> p h d", h=HPC),
                    )

        def normalize_finish(qt, avs_list, rec4):
            # bc matmuls + otmp muls + oT DMA for all 4 heads of qt; called
            # one stage later so the reciprocal (DVE) has long finished.
            for h in range(HPC):
                t, hh = h // 2, h % 2
                avs = avs_list[h]
                bc_ps = ps_av.tile([64, QT], F32, tag="av", name="bc")
                nc.tensor.matmul(bc_ps[:, :], lhsT=eye4[:, h, :],
                                 rhs=rec4[:, :], start=True, stop=True)
                bc_sb = opool.tile([64, QT], F32, tag="bc")
                nc.scalar.activation(out=bc_sb, in_=bc_ps, func=AF.Copy)
                otmp = opool.tile([64, QT], BF16, tag="otmp")
                nc.vector.tensor_mul(otmp, avs[0:DK, :], bc_sb)
                nc.gpsimd.dma_start(
                    out=oT[t][64 * hh:64 * hh + 64, qt * QT:(qt + 1) * QT],
                    in_=otmp)

        def outproj_jb(qt, jb, pool, tag):
            po = pool.tile([128, QT], F32, tag=tag, name="po") if tag == "av" \
                else pool.tile([128, 2, QT], F32, tag=tag, name="po")[:, 0, :]
            for mb in range(2):
                nc.tensor.matmul(
                    po[:, :],
                    lhsT=woT_sb[:, mb, jb * 128:(jb + 1) * 128],
                    rhs=oT[mb][:, qt * QT:(qt + 1) * QT],
                    start=(mb == 0), stop=(mb == 1),
                )
            ob = opool.tile([128, QT], F32, tag="ob")
            nc.vector.tensor_copy(ob, po)
            nc.sync.dma_start(
                out=out_d[jb * 128:(jb + 1) * 128, qt * QT:(qt + 1) * QT],
                in_=ob)

        def outproj(qt):
            for jb in range(8):
                outproj_jb(qt, jb, ps_big, "big")

        def attention_hp(qt, hp, av2, filler):
            nkb = 4 * qt + 4 if causal else NKB
            pending_av = []   # [(kb, c0, pt2)] awaiting AV matmuls

            def flush_av(last, depth=0):
                while len(pending_av) > depth:
                    pkb, pc0, ppt2 = pending_av.pop(0)
                    for j in range(2):
                        h = 2 * hp + j
                        nc.tensor.matmul(
                            av2[j][:, pc0:QT],
                            lhsT=v_sb[:, pkb, h, :],
                            rhs=ppt2[:, j, pc0:QT],
                            start=(pkb == 0),
                            stop=(last and not pending_av),
                        )

            nfil = len(filler)
            nspread = max(1, (nkb * 4) // 5)   # finish fillers by ~80% in
            for kb in range(nkb):
                lo = min(kb, nspread) * nfil // nspread
                hi = min(kb + 1, nspread) * nfil // nspread
                for f in filler[lo:hi]:
                    f()
                diag = causal and (kb >= 4 * qt)
                c0 = 128 * (kb - 4 * qt) if diag else 0
                sc2 = ps_big.tile([128, 2, QT], F32, tag="big", name="sc2")
                if not causal:
                    am = ampool.tile([KB, QT], F32, tag="am")
                    nc.sync.dma_start(
                        out=am,
                        in_=pat_d[kb * KB:(kb + 1) * KB,
                                  qt * QT:(qt + 1) * QT])
                for j in range(2):
                    h = 2 * hp + j
                    t, hh = h // 2, h % 2
                    nc.tensor.matmul(
                        sc2[:, j, c0:QT],
                        lhsT=kb_t[t][64 * hh:64 * hh + 64,
                                     kb * KB:(kb + 1) * KB],
                        rhs=qb[t][64 * hh:64 * hh + 64,
                                  qt * QT + c0:(qt + 1) * QT],
                        start=True, stop=True,
                    )
                    if causal and diag:
                        # mask the triangular [128,128] sub-block in place
                        nc.vector.scalar_tensor_tensor(
                            out=sc2[:, j, c0:c0 + KB],
                            in0=sc2[:, j, c0:c0 + KB],
                            scalar=0.0, in1=addtri,
                            op0=OP.add, op1=OP.add)
                    elif not causal:
                        nc.vector.scalar_tensor_tensor(
                            out=sc2[:, j, :], in0=sc2[:, j, :],
                            scalar=0.0, in1=am,
                            op0=OP.add, op1=OP.add)
                pt2 = ppool.tile([128, 2, QT], BF16, tag="pt")
                nc.scalar.activation(
                    out=pt2[:, :, c0:QT], in_=sc2[:, :, c0:QT],
                    func=AF.Exp, scale=0.125)
                pending_av.append((kb, c0, pt2))
                flush_av(last=False, depth=3)
            flush_av(last=True)

        # ---- emission: prologue proj(0), then per qt the attention stream
        # with next-tile projection matmuls interleaved as PE filler ----
        nc.sync.dma_start(out=wq, in_=wqT_d[:])
        pairs, fin = make_qkproj(xqT_d, wq, bq_sb, qb, 0)
        nc.sync.dma_start(out=wk, in_=wkT_d[:])
        for p in pairs:
            p()
        fin()
        pairs, fin = make_qkproj(xkT_d, wk, bk_sb, kb_t, 0)
        nc.sync.dma_start(out=wv, in_=wvT_d[:])
        for p in pairs:
            p()
        fin()
        proj_v(0)

        norm_state = {}
        for st in range(NQT):
            if st >= 1:
                normalize_finish(st - 1, *norm_state[st - 1])
            avs_list = []
            for hp in range(2):
                if st < NQT - 1:
                    if hp == 0:
                        filler, fin = make_qkproj(xqT_d, wq, bq_sb, qb, st + 1)
                    else:
                        filler, fin = make_qkproj(xkT_d, wk, bk_sb, kb_t, st + 1)
                    filler = filler + [fin]
                    fin = None
                else:
                    # last tile: earlier tiles' output projections are the
                    # PE filler (their oT finished stages ago)
                    pqt = NQT - 3 if hp == 0 else NQT - 2
                    filler = [(lambda jb=jb, pqt=pqt:
                               outproj_jb(pqt, jb, ps_av, "av"))
                              for jb in range(8)]
                    fin = None
                av2 = [ps_av.tile([DK + 1, QT], F32, tag="av", name=f"av{j}")
                       for j in range(2)]
                attention_hp(st, hp, av2, filler)
                for j in range(2):
                    avs = opool.tile([65, QT], F32, tag="avs", bufs=5)
                    nc.vector.tensor_copy(avs, av2[j])
                    avs_list.append(avs)
                if fin is not None:
                    fin()
                if hp == 0 and st == 1:
                    outproj(st - 1)
            lrow4 = opool.tile([4, QT], F32, tag="lrow4", bufs=2)
            for h in range(HPC):
                nc.gpsimd.dma_start(out=lrow4[h:h + 1, :],
                                    in_=avs_list[h][64:65, :])
            # rec4 padded to 128 partitions (zeros) so the bc matmul reads
            # full SBUF bandwidth (K_c=4 was ~4x slower than K_c=128)
            rec4 = opool.tile([128, QT], F32, tag="rec4", bufs=2)
            nc.gpsimd.memset(rec4, 0.0)
            nc.vector.reciprocal(rec4[0:4, :], lrow4)
            norm_state[st] = (avs_list, rec4)
            if st < NQT - 1:
                proj_v(st + 1)
        normalize_finish(NQT - 1, *norm_state[NQT - 1])
        outproj(NQT - 1)

    with tile.TileContext(nc) as tc:
        emit(tc)
    nc.compile()
    return nc


def _host_prep(inputs):
    k, q, v = inputs["k"], inputs["q"], inputs["v"]
    mask, sin, cos = inputs["mask"], inputs["sin"], inputs["cos"]
    Wq, bq = inputs["Wq"], inputs["bq"]
    Wk, bk = inputs["Wk"], inputs["bk"]
    Wv, bv = inputs["Wv"], inputs["bv"]
    Wo = inputs["Wo"]

    causal = bool(np.array_equal(
        np.asarray(mask[0, 0]),
        np.tril(np.ones((S, S), np.asarray(mask).dtype))))

    sinT = np.ascontiguousarray(np.asarray(sin[0, 0]).T.astype(np.float32))
    cosT = np.ascontiguousarray(np.asarray(cos[0, 0]).T.astype(np.float32))
    sin4 = np.ascontiguousarray(np.tile(sinT, (4, 1)))
    cos4 = np.ascontiguousarray(np.tile(cosT, (4, 1)))

    if causal:
        ki = np.arange(KB)[:, None]
        qi = np.arange(KB)[None, :]
        pat = np.ascontiguousarray(
            np.where(ki <= qi, 0.0, NEG8).astype(np.float32))
        amaskT = None
    else:
        pat = None
        amaskT = np.ascontiguousarray(
            np.where(np.asarray(mask[0, 0]).T == 0, NEG8, 0.0).astype(np.float32))

    # E/O permutation of the 256 local head dims:
    # [all heads' even dims | all heads' odd dims]
    ev, od = np.arange(0, DK, 2), np.arange(1, DK, 2)
    perm = np.concatenate(
        [64 * h + ev for h in range(4)] + [64 * h + od for h in range(4)])

    eye4 = np.zeros((128, 4, 64), np.float32)
    for h in range(4):
        eye4[h, h, :] = 1.0

    xT = {}
    for name, x in (("q", q), ("k", k), ("v", v)):
        for b in range(B):
            xt = np.asarray(x[b]).T.astype(BF)          # [D, S]
            if name == "v":
                t = xt.reshape(NDB, 128, 2 * NQT, 256).transpose(2, 1, 0, 3)
            else:
                t = xt.reshape(NDB, 128, NQT, QT).transpose(2, 1, 0, 3)
            xT[(name, b)] = np.ascontiguousarray(t)

    in_maps = []
    for c in range(NCORES):
        b, hg = c // 4, c % 4
        rows = slice(hg * DH, (hg + 1) * DH)
        def wtile(w):
            return np.ascontiguousarray(
                w.astype(np.float32).reshape(NDB, 128, DH).transpose(1, 0, 2))
        wqT = wtile(np.asarray(Wq)[rows, :].T[:, perm])
        wkT = wtile(np.asarray(Wk)[rows, :].T[:, perm])
        wvT = wtile(np.asarray(Wv)[rows, :].T)
        woT = np.ascontiguousarray(np.asarray(Wo)[:, rows].T
                                   .astype(np.float32)
                                   .reshape(2, 128, D).transpose(1, 0, 2))
        m = dict(
            xqT=xT[("q", b)], xkT=xT[("k", b)], xvT=xT[("v", b)],
            wqT=wqT.astype(BF), wkT=wkT.astype(BF),
            wvT=wvT.astype(BF), woT=woT.astype(BF),
            bqA=np.ascontiguousarray(np.asarray(bq)[rows][perm]).astype(np.float32),
            bkA=np.ascontiguousarray(np.asarray(bk)[rows][perm]).astype(np.float32),
            bv=np.ascontiguousarray(np.asarray(bv)[rows]).astype(np.float32),
            sin4=sin4, cos4=cos4, eye4=eye4,
        )
        if causal:
            m["addtri"] = pat
        else:
            m["amaskT"] = amaskT
        in_maps.append(m)
    return causal, in_maps


def kernel(**inputs):
    from concourse.bass_utils import run_bass_kernel_spmd

    causal, in_maps = _host_prep(inputs)
    if causal not in _cache:
        _cache[causal] = _build_nc(causal)
    nc = _cache[causal]

    res = run_bass_kernel_spmd(nc, in_maps, core_ids=list(range(NCORES))).results

    bo = np.asarray(inputs["bo"]).astype(np.float32)
    out = np.empty((B, S, D), np.float32)
    for b in range(B):
        acc = res[4 * b]["outT"].astype(np.float32).copy()
        for c in range(4 * b + 1, 4 * b + 4):
            acc += res[c]["outT"]
        out[b] = acc.T + bo
    return out


# revision 31
# speedup vs baseline: 1.0774x; 1.0208x over previous
"""Multi-head attention (B=2, S=2048, D=1024, H=16, DK=64) with RoPE and
causal masking, sharded over 8 NeuronCores as (batch x head-group):
core c handles batch c//4 and heads 4*(c%4) .. 4*(c%4)+4.

Per-core dataflow (all layouts chosen so no on-device transposes are needed):
  - host pre-transposes activations X^T [D, S] and weight slices.
  - projections produce Q^T/K^T in an "A layout" per 128-partition tile
    ([h0e|h1e|h0o|h1o]: RoPE even/odd dims grouped) via PE matmul,
    evicted from PSUM with fused bias add (DVE tensor_scalar_add).
  - RoPE runs full-width DVE ops on the A tiles and re-packs into the
    "B layout" ([h0e'|h0o'|h1e'|h1o']: head-contiguous, 64 dims/head).
  - scores_t[k,q] = K_B.T @ Q_B per (qtile 512, kblock 128, head), K_c=64.
    Softmax without max-subtraction (scores ~ N(0,1), safe in fp32):
    P = exp(0.125*s + addpat) where addpat is 0/-1e30; causal blocks above
    the diagonal are skipped entirely.
  - AV: lhsT = [V | ones] (M=65) stationary, rhs = P_t moving; PSUM
    accumulates over kblocks; row 64 gives the softmax denominator l.
  - normalize: O^T = AV[0:64] * broadcast(1/l) (broadcast via K_c=1 matmul).
  - output projection: partial^T[j,s] = woT.T @ O^T; host sums the 4
    partials per batch, transposes, and adds bo.

The whole thing is emitted as one software pipeline over the 4 q s-tiles:
projections(st) -> RoPE(st) -> attention(qt=st) -> out-proj(st), so Tile
overlaps DMA/PE/DVE/ACT across phases.
"""
import numpy as np
import ml_dtypes

BF = ml_dtypes.bfloat16
B, S, D, H, DK = 2, 2048, 1024, 16, 64
NCORES = 8
HPC = 4            # heads per core
DH = HPC * DK      # 256 local head dims
QT = 512           # q tile (free dim of scores matmul)
KB = 128           # k block (partition dim of scores)
NQT = S // QT      # 4
NKB = S // KB      # 16
NDB = D // 128     # 8 d-blocks for projections
NEG = -1.0e30
NEG8 = -1.0e31   # pre-scale mask value: *0.125 -> -1.25e30

_cache = {}


def _build_nc(causal: bool):
    from contextlib import ExitStack
    import concourse.bass as bass
    import concourse.tile as tile
    from concourse import bacc, mybir
    from concourse._compat import with_exitstack

    F32 = mybir.dt.float32
    BF16 = mybir.dt.bfloat16
    AF = mybir.ActivationFunctionType
    OP = mybir.AluOpType

    nc = bacc.Bacc(None, target_bir_lowering=False, debug=False)

    xqT_d = nc.dram_tensor("xqT", [NQT, 128, NDB, QT], BF16, kind="ExternalInput")
    xkT_d = nc.dram_tensor("xkT", [NQT, 128, NDB, QT], BF16, kind="ExternalInput")
    xvT_d = nc.dram_tensor("xvT", [2 * NQT, 128, NDB, 256], BF16, kind="ExternalInput")
    wqT_d = nc.dram_tensor("wqT", [128, NDB, DH], BF16, kind="ExternalInput")
    wkT_d = nc.dram_tensor("wkT", [128, NDB, DH], BF16, kind="ExternalInput")
    wvT_d = nc.dram_tensor("wvT", [128, NDB, DH], BF16, kind="ExternalInput")
    bqA_d = nc.dram_tensor("bqA", [DH], F32, kind="ExternalInput")
    bkA_d = nc.dram_tensor("bkA", [DH], F32, kind="ExternalInput")
    bv_d = nc.dram_tensor("bv", [DH], F32, kind="ExternalInput")
    woT_d = nc.dram_tensor("woT", [128, 2, D], BF16, kind="ExternalInput")
    sin_d = nc.dram_tensor("sin4", [128, S], F32, kind="ExternalInput")
    cos_d = nc.dram_tensor("cos4", [128, S], F32, kind="ExternalInput")
    if causal:
        pat_d = nc.dram_tensor("addtri", [KB, KB], F32, kind="ExternalInput")
    else:
        pat_d = nc.dram_tensor("amaskT", [S, S], F32, kind="ExternalInput")
    eye4_d = nc.dram_tensor("eye4", [128, 4, 64], F32, kind="ExternalInput")
    out_d = nc.dram_tensor("outT", [D, S], F32, kind="ExternalOutput")

    @with_exitstack
    def emit(ctx: ExitStack, tc: tile.TileContext):
        nc = tc.nc
        singles = ctx.enter_context(tc.tile_pool(name="singles", bufs=1))
        xpool = ctx.enter_context(tc.tile_pool(name="x", bufs=3))
        ropet = ctx.enter_context(tc.tile_pool(name="ropet", bufs=2))
        bpool = ctx.enter_context(tc.tile_pool(name="bpool", bufs=1))
        ppool = ctx.enter_context(tc.tile_pool(name="ppool", bufs=8))
        opool = ctx.enter_context(tc.tile_pool(name="opool", bufs=3))
        if not causal:
            ampool = ctx.enter_context(tc.tile_pool(name="ampool", bufs=3))

        # "big" slots are 2 banks each ([128,2,QT]); proj psum pairs and
        # score groups rotate through them. av holds AV accumulators and the
        # tiny broadcast matmuls.
        ps_big = ctx.enter_context(tc.tile_pool(name="ps_big", bufs=2, space="PSUM"))
        ps_av = ctx.enter_context(tc.tile_pool(name="ps_av", bufs=4, space="PSUM"))

        # ---- constants on the gpsimd queue; weights go on sync, each
        # emitted right before its first consumer ----
        wq = singles.tile([128, NDB, DH], BF16)
        wk = singles.tile([128, NDB, DH], BF16)
        wv = singles.tile([128, NDB, DH], BF16)
        sin4 = singles.tile([128, S], F32)
        cos4 = singles.tile([128, S], F32)
        nc.gpsimd.dma_start(out=sin4, in_=sin_d[:])
        nc.gpsimd.dma_start(out=cos4, in_=cos_d[:])
        eye4 = singles.tile([128, 4, 64], F32)
        nc.gpsimd.dma_start(out=eye4, in_=eye4_d[:])
        if causal:
            addtri = singles.tile([KB, KB], F32)
            nc.gpsimd.dma_start(out=addtri, in_=pat_d[:])

        bq_sb = singles.tile([128, 2], F32)
        bk_sb = singles.tile([128, 2], F32)
        nc.gpsimd.dma_start(out=bq_sb, in_=bqA_d.rearrange("(t p) -> p t", p=128))
        nc.gpsimd.dma_start(out=bk_sb, in_=bkA_d.rearrange("(t p) -> p t", p=128))
        bv_bc = singles.tile([128, DH], F32)
        nc.gpsimd.dma_start(
            out=bv_bc,
            in_=bass.AP(tensor=bv_d[:].tensor, offset=0, ap=[[0, 128], [1, DH]]),
        )

        woT_sb = singles.tile([128, 2, D], BF16)
        nc.gpsimd.dma_start(out=woT_sb, in_=woT_d[:])

        # persistent activations (post-RoPE Q/K and attention output, T layout)
        qb = [bpool.tile([128, S], BF16, tag=f"qb{t}", name=f"qb{t}") for t in range(2)]
        kb_t = [bpool.tile([128, S], BF16, tag=f"kbt{t}", name=f"kbt{t}") for t in range(2)]
        oT = [bpool.tile([128, S], BF16, tag=f"oT{t}", name=f"oT{t}") for t in range(2)]
        v_sb = singles.tile([128, NKB, HPC, DK + 1], BF16)
        nc.vector.memset(v_sb[:, :, :, DK:DK + 1], 1.0)

        def make_qkproj(xd, ws, bias_sb, dst, st):
            """Returns (pairs, finish): `pairs` are per-d-block closures each
            emitting the E+O matmuls (interleaved into the attention stream
            as PE keep-alive filler); `finish` emits bias+RoPE (STT) and the
            stream-order scatter DMA into the head-contiguous dst tiles."""
            xt = [xpool.tile([128, NDB // 2, QT], BF16, tag="xtA", name="xtA"),
                  xpool.tile([128, NDB // 2, QT], BF16, tag="xtB", name="xtB")]
            for hlf in range(2):
                nc.sync.dma_start(
                    out=xt[hlf],
                    in_=xd[st, :, hlf * (NDB // 2):(hlf + 1) * (NDB // 2), :])
            psE = ps_av.tile([128, QT], F32, tag="av", name="pE")
            psO = ps_av.tile([128, QT], F32, tag="av", name="pO")

            def pair(db):
                def go():
                    for mt, ps in ((0, psE), (1, psO)):
                        nc.tensor.matmul(
                            ps[:, :],
                            lhsT=ws[:, db, mt * 128:(mt + 1) * 128],
                            rhs=xt[db // 4][:, db % 4, :],
                            start=(db == 0), stop=(db == NDB - 1),
                        )
                return go

            def finish():
                sl = slice(st * QT, (st + 1) * QT)
                t1 = ropet.tile([128, QT], F32, tag="t1")
                t2 = ropet.tile([128, QT], F32, tag="t2")
                t3 = ropet.tile([128, QT], F32, tag="t3")
                t4 = ropet.tile([128, QT], F32, tag="t4")
                nc.vector.scalar_tensor_tensor(
                    out=t1, in0=psE, scalar=bias_sb[:, 0:1],
                    in1=sin4[:, sl], op0=OP.add, op1=OP.mult)
                nc.vector.scalar_tensor_tensor(
                    out=t3, in0=psE, scalar=bias_sb[:, 0:1],
                    in1=cos4[:, sl], op0=OP.add, op1=OP.mult)
                nc.vector.scalar_tensor_tensor(
                    out=t2, in0=psO, scalar=bias_sb[:, 1:2],
                    in1=cos4[:, sl], op0=OP.add, op1=OP.mult)
                nc.vector.scalar_tensor_tensor(
                    out=t4, in0=psO, scalar=bias_sb[:, 1:2],
                    in1=sin4[:, sl], op0=OP.add, op1=OP.mult)
                eo = ropet.tile([128, 2, QT], BF16, tag="eo")
                nc.vector.tensor_sub(eo[:, 0, :], t3, t4)    # evens'
                nc.vector.tensor_add(eo[:, 1, :], t1, t2)    # odds'
                # one DMA per dst tile: the element stream (partition-major,
                # then [eo, q]) lands as rows [e0,o0,e1,o1,...] per head --
                # natural interleaved head-contiguous layout, same for Q & K.
                for t in range(2):
                    nc.gpsimd.dma_start(
                        out=dst[t][:, sl],
                        in_=eo[64 * t:64 * t + 64, :, :])
            return [pair(db) for db in range(NDB)], finish

        def proj_v(st):
            # V rows [s in st*QT..(st+1)*QT) -> kblocks 4st..4st+4 (ps_big)
            for half in range(2):
                s0 = st * QT + half * 256
                xv = xpool.tile([128, NDB, 256], BF16, tag="xv")
                nc.sync.dma_start(out=xv, in_=xvT_d[2 * st + half])
                ps2 = ps_big.tile([128, 2, QT], F32, tag="big", name="pv")
                ps = [ps2[:, 0, 0:DH], ps2[:, 1, 0:DH]]
                for db in range(NDB):
                    for kbl in range(2):
                        nc.tensor.matmul(
                            ps[kbl][:, :],
                            lhsT=xv[:, db, kbl * 128:(kbl + 1) * 128],
                            rhs=wv[:, db, :],
                            start=(db == 0), stop=(db == NDB - 1),
                        )
                for kbl in range(2):
                    kb = s0 // 128 + kbl
                    nc.vector.tensor_add(
                        v_sb[:, kb, :, 0:DK],
                        ps[kbl].rearrange("p (h d) -> p h d", h=HPC),
                        bv_bc.rearrange("p (h d) -> p h d", h=HPC),
                    )

        def normalize_finish(qt, avs_list, rec4):
            # bc matmuls + otmp muls + oT DMA for all 4 heads of qt; called
            # one stage later so the reciprocal (DVE) has long finished.
            for h in range(HPC):
                t, hh = h // 2, h % 2
                avs = avs_list[h]
                bc_ps = ps_av.tile([64, QT], F32, tag="av", name="bc")
                nc.tensor.matmul(bc_ps[:, :], lhsT=eye4[:, h, :],
                                 rhs=rec4[:, :], start=True, stop=True)
                bc_sb = opool.tile([64, QT], F32, tag="bc")
                nc.scalar.activation(out=bc_sb, in_=bc_ps, func=AF.Copy)
                otmp = opool.tile([64, QT], BF16, tag="otmp")
                nc.vector.tensor_mul(otmp, avs[0:DK, :], bc_sb)
                nc.gpsimd.dma_start(
                    out=oT[t][64 * hh:64 * hh + 64, qt * QT:(qt + 1) * QT],
                    in_=otmp)

        def outproj_jb(qt, jb, pool, tag):
            po = pool.tile([128, QT], F32, tag=tag, name="po") if tag == "av" \
                else pool.tile([128, 2, QT], F32, tag=tag, name="po")[:, 0, :]
            for mb in range(2):
                nc.tensor.matmul(
                    po[:, :],
                    lhsT=woT_sb[:, mb, jb * 128:(jb + 1) * 128],
                    rhs=oT[mb][:, qt * QT:(qt + 1) * QT],
                    start=(mb == 0), stop=(mb == 1),
                )
            ob = opool.tile([128, QT], F32, tag="ob")
            nc.vector.tensor_copy(ob, po)
            nc.sync.dma_start(
                out=out_d[jb * 128:(jb + 1) * 128, qt * QT:(qt + 1) * QT],
                in_=ob)

        def outproj(qt):
            for jb in range(8):
                outproj_jb(qt, jb, ps_big, "big")

        def attention_hp(qt, hp, av2, filler):
            nkb = 4 * qt + 4 if causal else NKB
            pending_av = []   # [(kb, c0, pt2)] awaiting AV matmuls

            def flush_av(last, depth=0):
                while len(pending_av) > depth:
                    pkb, pc0, ppt2 = pending_av.pop(0)
                    for j in range(2):
                        h = 2 * hp + j
                        nc.tensor.matmul(
                            av2[j][:, pc0:QT],
                            lhsT=v_sb[:, pkb, h, :],
                            rhs=ppt2[:, j, pc0:QT],
                            start=(pkb == 0),
                            stop=(last and not pending_av),
                        )

            nfil = len(filler)
            nspread = max(1, (nkb * 4) // 5)   # finish fillers by ~80% in
            for kb in range(nkb):
                lo = min(kb, nspread) * nfil // nspread
                hi = min(kb + 1, nspread) * nfil // nspread
                for f in filler[lo:hi]:
                    f()
                diag = causal and (kb >= 4 * qt)
                c0 = 128 * (kb - 4 * qt) if diag else 0
                sc2 = ps_big.tile([128, 2, QT], F32, tag="big", name="sc2")
                if not causal:
                    am = ampool.tile([KB, QT], F32, tag="am")
                    nc.sync.dma_start(
                        out=am,
                        in_=pat_d[kb * KB:(kb + 1) * KB,
                                  qt * QT:(qt + 1) * QT])
                for j in range(2):
                    h = 2 * hp + j
                    t, hh = h // 2, h % 2
                    nc.tensor.matmul(
                        sc2[:, j, c0:QT],
                        lhsT=kb_t[t][64 * hh:64 * hh + 64,
                                     kb * KB:(kb + 1) * KB],
                        rhs=qb[t][64 * hh:64 * hh + 64,
                                  qt * QT + c0:(qt + 1) * QT],
                        start=True, stop=True,
                    )
                    if causal and diag:
                        # mask the triangular [128,128] sub-block in place
                        nc.vector.scalar_tensor_tensor(
                            out=sc2[:, j, c0:c0 + KB],
                            in0=sc2[:, j, c0:c0 + KB],
                            scalar=0.0, in1=addtri,
                            op0=OP.add, op1=OP.add)
                    elif not causal:
                        nc.vector.scalar_tensor_tensor(
                            out=sc2[:, j, :], in0=sc2[:, j, :],
                            scalar=0.0, in1=am,
                            op0=OP.add, op1=OP.add)
                pt2 = ppool.tile([128, 2, QT], BF16, tag="pt")
                nc.scalar.activation(
                    out=pt2[:, :, c0:QT], in_=sc2[:, :, c0:QT],
                    func=AF.Exp, scale=0.125)
                pending_av.append((kb, c0, pt2))
                flush_av(last=False, depth=3)
            flush_av(last=True)

        # ---- emission: prologue proj(0), then per qt the attention stream
        # with next-tile projection matmuls interleaved as PE filler ----
        nc.sync.dma_start(out=wq, in_=wqT_d[:])
        pairs, fin = make_qkproj(xqT_d, wq, bq_sb, qb, 0)
        nc.sync.dma_start(out=wk, in_=wkT_d[:])
        for p in pairs:
            p()
        fin()
        pairs, fin = make_qkproj(xkT_d, wk, bk_sb, kb_t, 0)
        nc.sync.dma_start(out=wv, in_=wvT_d[:])
        for p in pairs:
            p()
        fin()
        proj_v(0)

        norm_state = {}
        for st in range(NQT):
            if st >= 1:
                normalize_finish(st - 1, *norm_state[st - 1])
            avs_list = []
            for hp in range(2):
                if st < NQT - 1:
                    if hp == 0:
                        filler, fin = make_qkproj(xqT_d, wq, bq_sb, qb, st + 1)
                    else:
                        filler, fin = make_qkproj(xkT_d, wk, bk_sb, kb_t, st + 1)
                    filler = filler + [fin]
                    fin = None
                else:
                    # last tile: earlier tiles' output projections are the
                    # PE filler (their oT finished stages ago)
                    pqt = NQT - 3 if hp == 0 else NQT - 2
                    filler = [(lambda jb=jb, pqt=pqt:
                               outproj_jb(pqt, jb, ps_av, "av"))
                              for jb in range(8)]
                    fin = None
                av2 = [ps_av.tile([DK + 1, QT], F32, tag="av", name=f"av{j}")
                       for j in range(2)]
                attention_hp(st, hp, av2, filler)
                for j in range(2):
                    avs = opool.tile([65, QT], F32, tag="avs", bufs=5)
                    nc.vector.tensor_copy(avs, av2[j])
                    avs_list.append(avs)
                if fin is not None:
                    fin()
                if hp == 0 and st == 1:
                    outproj(st - 1)
            lrow4 = opool.tile([4, QT], F32, tag="lrow4", bufs=2)
            for h in range(HPC):
                nc.gpsimd.dma_start(out=lrow4[h:h + 1, :],
                                    in_=avs_list[h][64:65, :])
            # rec4 padded to 128 partitions (zeros) so the bc matmul reads
            # full SBUF bandwidth (K_c=4 was ~4x slower than K_c=128)
            rec4 = opool.tile([128, QT], F32, tag="rec4", bufs=2)
            nc.gpsimd.memset(rec4, 0.0)
            nc.vector.reciprocal(rec4[0:4, :], lrow4)
            norm_state[st] = (avs_list, rec4)
            if st < NQT - 1:
                proj_v(st + 1)
        normalize_finish(NQT - 1, *norm_state[NQT - 1])
        outproj(NQT - 1)

    with tile.TileContext(nc) as tc:
        emit(tc)
    nc.compile()
    return nc


def _host_prep(inputs):
    k, q, v = inputs["k"], inputs["q"], inputs["v"]
    mask, sin, cos = inputs["mask"], inputs["sin"], inputs["cos"]
    Wq, bq = inputs["Wq"], inputs["bq"]
    Wk, bk = inputs["Wk"], inputs["bk"]
    Wv, bv = inputs["Wv"], inputs["bv"]
    Wo = inputs["Wo"]

    causal = bool(np.array_equal(
        np.asarray(mask[0, 0]),
        np.tril(np.ones((S, S), np.asarray(mask).dtype))))

    sinT = np.ascontiguousarray(np.asarray(sin[0, 0]).T.astype(np.float32))
    cosT = np.ascontiguousarray(np.asarray(cos[0, 0]).T.astype(np.float32))
    sin4 = np.ascontiguousarray(np.tile(sinT, (4, 1)))
    cos4 = np.ascontiguousarray(np.tile(cosT, (4, 1)))

    if causal:
        ki = np.arange(KB)[:, None]
        qi = np.arange(KB)[None, :]
        pat = np.ascontiguousarray(
            np.where(ki <= qi, 0.0, NEG8).astype(np.float32))
        amaskT = None
    else:
        pat = None
        amaskT = np.ascontiguousarray(
            np.where(np.asarray(mask[0, 0]).T == 0, NEG8, 0.0).astype(np.float32))

    # E/O permutation of the 256 local head dims:
    # [all heads' even dims | all heads' odd dims]
    ev, od = np.arange(0, DK, 2), np.arange(1, DK, 2)
    perm = np.concatenate(
        [64 * h + ev for h in range(4)] + [64 * h + od for h in range(4)])

    eye4 = np.zeros((128, 4, 64), np.float32)
    for h in range(4):
        eye4[h, h, :] = 1.0

    xT = {}
    for name, x in (("q", q), ("k", k), ("v", v)):
        for b in range(B):
            xt = np.asarray(x[b]).T.astype(BF)          # [D, S]
            if name == "v":
                t = xt.reshape(NDB, 128, 2 * NQT, 256).transpose(2, 1, 0, 3)
            else:
                t = xt.reshape(NDB, 128, NQT, QT).transpose(2, 1, 0, 3)
            xT[(name, b)] = np.ascontiguousarray(t)

    in_maps = []
    for c in range(NCORES):
        b, hg = c // 4, c % 4
        rows = slice(hg * DH, (hg + 1) * DH)
        def wtile(w):
            return np.ascontiguousarray(
                w.astype(np.float32).reshape(NDB, 128, DH).transpose(1, 0, 2))
        wqT = wtile(np.asarray(Wq)[rows, :].T[:, perm])
        wkT = wtile(np.asarray(Wk)[rows, :].T[:, perm])
        wvT = wtile(np.asarray(Wv)[rows, :].T)
        woT = np.ascontiguousarray(np.asarray(Wo)[:, rows].T
                                   .astype(np.float32)
                                   .reshape(2, 128, D).transpose(1, 0, 2))
        m = dict(
            xqT=xT[("q", b)], xkT=xT[("k", b)], xvT=xT[("v", b)],
            wqT=wqT.astype(BF), wkT=wkT.astype(BF),
            wvT=wvT.astype(BF), woT=woT.astype(BF),
            bqA=np.ascontiguousarray(np.asarray(bq)[rows][perm]).astype(np.float32),
            bkA=np.ascontiguousarray(np.asarray(bk)[rows][perm]).astype(np.float32),
            bv=np.ascontiguousarray(np.asarray(bv)[rows]).astype(np.float32),
            sin4=sin4, cos4=cos4, eye4=eye4,
        )
        if causal:
            m["addtri"] = pat
        else:
            m["amaskT"] = amaskT
        in_maps.append(m)
    return causal, in_maps


def kernel(**inputs):
    from concourse.bass_utils import run_bass_kernel_spmd

    causal, in_maps = _host_prep(inputs)
    if causal not in _cache:
        _cache[causal] = _build_nc(causal)
    nc = _cache[causal]

    res = run_bass_kernel_spmd(nc, in_maps, core_ids=list(range(NCORES))).results

    bo = np.asarray(inputs["bo"]).astype(np.float32)
    out = np.empty((B, S, D), np.float32)
    for b in range(B):
        acc = res[4 * b]["outT"].astype(np.float32).copy()
        for c in range(4 * b + 1, 4 * b + 4):
            acc += res[c]["outT"]
        out[b] = acc.T + bo
    return out
